# revision 54
# baseline (speedup 1.0000x reference)
"""GQA kernel for trn2, 8 NeuronCores — bf16 PE pipeline.

Problem: B=2, N=2048, d_model=2048, 32 q heads / 8 kv heads, d_head=64.
Sharding: batch (2) x head-groups (4): core c = b*4 + g handles batch b and
q heads [8g, 8g+8) (kv heads [2g, 2g+1]).  Each core computes
partial_out = attn_out_g @ Wo[:, cols_g].T ; host sums the 4 group partials
per batch and adds bo.

Key speed choices vs the fp32 baseline:
  * all matmuls in bf16 (1 cycle/row vs 4 for fp32 on the PE)
  * x is transposed to xT on the host -> no on-device transpose phase
  * V' carries a ones column (65th) so the softmax denominator falls out
    of the PV matmul as output partition 64 -- no separate denominator
    matmuls
  * Act engine runs only exp (+ K/Q bias copies); normalize reciprocal is
    broadcast across partitions by gpsimd and multiplied on DVE
  * S -> exp -> PV software-pipelined one key-chunk deep; out-proj and
    next-tile Q-proj matmuls are interleaved as Act-independent PE filler

Per-core layouts (bf16 unless noted):
  xT_sb [128, 16*2048]  d_model chunk ck at free [ck*2048, +2048); free=toks
  qT  [128, 4*2048]  chunk j holds heads (j, j+4): partitions 0:64 = head j
                     dims, 64:128 = head j+4 dims; free = tokens.
  kT  [128, 2048]    partitions 0:64 = kv0 k-dims, 64:128 = kv1 k-dims.
  vp0/vp1 [128, 16*65]  V' per kv head: partitions = keys (chunk kc at
                     free [kc*65, +65)), free = 64 v dims + ones col.
  S.T computed as [keys, q] (exp is safe unnormalized: |scores/8| < ~6).
"""

import numpy as np
import ml_dtypes

import concourse.bass as bass
import concourse.mybir as mybir
from concourse.tile import TileContext, add_dep_helper
from concourse.bass_utils import run_bass_kernel_spmd


def _split_matmul_waits(bir_bytes):
    """Walrus in this toolchain allows only ONE sync wait per Matmult.

    For any matmul carrying N>1 waits, insert a PE NoOp immediately
    before it holding the first N-1 waits; the matmul keeps the last.
    The NoOp precedes the matmul in the PE stream, so ordering
    semantics are identical.
    """
    import json as _json
    bir = _json.loads(bir_bytes)
    n = 0
    for f in bir["functions"]:
        for b in f["blocks"]:
            out = []
            for i in b["instructions"]:
                si = i.get("sync_info") if isinstance(i, dict) else None
                eng = i.get("engine") if isinstance(i, dict) else None
                if (si and len(si.get("on_wait", [])) > 1
                        and eng and eng != "Unassigned"):
                    waits = si["on_wait"]
                    for w in waits[:-1]:
                        out.append({
                            "debug": i.get("debug", 0),
                            "engine": eng,
                            "ins": [], "outs": [],
                            "name": "%s-w%d" % (i["name"], n),
                            "opcode": "NoOp",
                            "sync_info": {"on_update": [], "on_wait": [w]},
                        })
                        n += 1
                    si["on_wait"] = waits[-1:]
                out.append(i)
            b["instructions"] = out
    return _json.dumps(bir).encode()


def _pe_touch(nc, producers):
    """Advance PE's vector clock past each producer, one sem at a time.

    This walrus build allows at most ONE sync wait per Matmult; a matmul
    whose inputs come from two not-yet-observed semaphores fails codegen
    ("Too many sync wait commands").  A PE nop with a single sync dep
    absorbs one semaphore each, so real matmuls only ever add one wait.
    """
    for p in producers:
        n = nc.tensor.nop()
        add_dep_helper(n.ins, p.ins, sync=True, reason="pe-wait-absorber")


F32 = mybir.dt.float32
BF16 = mybir.dt.bfloat16
FP8 = mybir.dt.float8e4
AF = mybir.ActivationFunctionType
DBLROW = mybir.MatmulPerfMode.DoubleRow

D = 2048      # d_model
TOKS = 2048   # tokens per batch
QD = 512      # q dims per core
DH = 64
NCK = 16      # d_model chunks of 128
TT = 512      # token tile for projections
NTT = TOKS // TT
QTILE = 512
NQT = TOKS // QTILE
NKC = TOKS // 128   # key chunks of 128
VW = DH + 1   # V' chunk width incl. ones column
SCALE = DH ** -0.5  # 0.125

# local head order within a core: chunk j holds heads (j, j+4)
HEAD_ORDER = [0, 4, 1, 5, 2, 6, 3, 7]


def _build():
    nc = bass.Bass()
    # weights come pre-packed in SBUF layout (partition-major) so each loads
    # with a single DMA of long descriptors; DMA queues are descriptor-rate
    # bound, not bandwidth bound
    xT = nc.declare_dram_parameter("xT", [D, TOKS], BF16, isOutput=False)
    wqP = nc.declare_dram_parameter("wqP", [128, NCK * QD], BF16,
                                    isOutput=False)
    wkP = nc.declare_dram_parameter("wkP", [128, NCK * 128], BF16,
                                    isOutput=False)
    wvP = nc.declare_dram_parameter("wvP", [128, NCK * 128], BF16,
                                    isOutput=False)
    woP = nc.declare_dram_parameter("woP", [128, 4 * D], BF16, isOutput=False)
    bq4 = nc.declare_dram_parameter("bq4", [128, 4], F32, isOutput=False)
    bkT = nc.declare_dram_parameter("bkT", [128, 1], F32, isOutput=False)
    bvb = nc.declare_dram_parameter("bvb", [128, 128], F32, isOutput=False)
    out = nc.declare_dram_parameter("out", [TOKS, D], BF16, isOutput=True)

    with TileContext(nc) as tc:
        with tc.tile_pool(name="persist", bufs=1) as pp:
            wq_sb = pp.tile([128, NCK * QD], BF16, tag="wq")
            wk_sb = pp.tile([128, NCK * 128], BF16, tag="wk")
            wv_sb = pp.tile([128, NCK * 128], BF16, tag="wv")
            wo_sb = pp.tile([128, 4 * D], BF16, tag="wo")
            xT_sb = pp.tile([128, NCK * TOKS], BF16, tag="xTs")
            qT = pp.tile([128, 4 * TOKS], BF16, tag="qT")
            kT = pp.tile([128, TOKS], BF16, tag="kT")
            vp0 = pp.tile([128, NKC * VW], BF16, tag="vp0")
            vp1 = pp.tile([128, NKC * VW], BF16, tag="vp1")
            bq_sb = pp.tile([128, 4], F32, tag="bq")
            bk_sb = pp.tile([128, 1], F32, tag="bk")
            bvb_sb = pp.tile([128, 128], F32, tag="bvb")
            ones_b = pp.tile([65, DH], BF16, tag="ones")
            zer_sb = pp.tile([65, 1], F32, tag="zer")

            const_loads = []
            # weights needed earliest first: wk/wq (K and Q0 streams) loaded
            # per d_model chunk, interleaved with the xT chunks they pair
            # with, so the streaming projections chase the DMAs
            const_loads.append(nc.sync.dma_start(out=bk_sb[:, :], in_=bkT[:, :]))
            const_loads.append(nc.sync.dma_start(out=bq_sb[:, :], in_=bq4[:, :]))
            for ck in range(NCK):
                const_loads.append(nc.sync.dma_start(
                    out=wk_sb[:, ck * 128:(ck + 1) * 128],
                    in_=wkP[:, ck * 128:(ck + 1) * 128]))
                const_loads.append(nc.sync.dma_start(
                    out=wq_sb[:, ck * QD:(ck + 1) * QD],
                    in_=wqP[:, ck * QD:(ck + 1) * QD]))
                const_loads.append(nc.sync.dma_start(
                    out=xT_sb[:, ck * TOKS:(ck + 1) * TOKS],
                    in_=xT[ck * 128:(ck + 1) * 128, :]))
            const_loads.append(nc.sync.dma_start(out=bvb_sb[:, :], in_=bvb[:, :]))
            for q in range(4):
                const_loads.append(nc.sync.dma_start(
                    out=wv_sb[:, q * 512:(q + 1) * 512],
                    in_=wvP[:, q * 512:(q + 1) * 512]))
            for j in range(4):
                const_loads.append(nc.sync.dma_start(
                    out=wo_sb[:, j * D:(j + 1) * D],
                    in_=woP[:, j * D:(j + 1) * D]))
            # ones columns of V': preset whole vp tiles to 1.0; the V adds
            # later overwrite the 64 data columns of each chunk
            const_loads.append(nc.vector.memset(vp0[:, :], 1.0))
            const_loads.append(nc.vector.memset(vp1[:, :], 1.0))
            const_loads.append(nc.vector.memset(ones_b[:, :], 1.0))
            const_loads.append(nc.vector.memset(zer_sb[:, :], 0.0))
            _pe_touch(nc, const_loads)

            # ---------------- K and first-tile-Q projections ----------------
            # Streamed per d_model chunk (outer ck) so the matmuls chase the
            # xT DMAs chunk-by-chunk instead of waiting for the whole load;
            # all 4 token-tile accumulators of each live concurrently.
            kv_tail = []
            with tc.tile_pool(name="kqps", bufs=1, space="PSUM") as kqp:
                kps, qps = [], []
                for tt in range(NTT):
                    kp_t = kqp.tile([128, TT], F32, tag="kp", bufs=NTT,
                                    name="kp")
                    qp_t = kqp.tile([128, TT], F32, tag="qp0", bufs=NTT,
                                    name="qp0")
                    kps.append(kp_t)
                    qps.append(qp_t)
                for ck in range(NCK):
                    for tt in range(NTT):
                        nc.tensor.matmul(
                            kps[tt][:, :],
                            wk_sb[:, ck * 128:(ck + 1) * 128],
                            xT_sb[:, ck * TOKS + tt * TT:
                                  ck * TOKS + (tt + 1) * TT],
                            start=(ck == 0), stop=(ck == NCK - 1))
                        nc.tensor.matmul(
                            qps[tt][:, :],
                            wq_sb[:, ck * QD + tt * 128:
                                  ck * QD + (tt + 1) * 128],
                            xT_sb[:, ck * TOKS:ck * TOKS + TT],
                            start=(ck == 0), stop=(ck == NCK - 1))
                for tt in range(NTT):
                    kv_tail.append(nc.scalar.activation(
                        kT[:, tt * TT:(tt + 1) * TT], kps[tt][:, :],
                        AF.Identity, bias=bk_sb[:, 0:1]))
                    kv_tail.append(nc.vector.tensor_scalar_add(
                        qT[:, tt * TOKS:tt * TOKS + TT],
                        qps[tt][:, :], bq_sb[:, tt:tt + 1]))
            # ---------------- V projection ----------------
            with tc.tile_pool(name="vps", bufs=1, space="PSUM") as kvp:
                for kc in range(NKC):
                    vch = kvp.tile([128, 128], F32, tag="vch", bufs=4)
                    for ck in range(NCK):
                        nc.tensor.matmul(
                            vch[:, :],
                            xT_sb[:, ck * TOKS + kc * 128:
                                  ck * TOKS + (kc + 1) * 128],
                            wv_sb[:, ck * 128:(ck + 1) * 128],
                            start=(ck == 0), stop=(ck == NCK - 1))
                    kv_tail.append(nc.vector.tensor_add(
                        vp0[:, kc * VW:kc * VW + DH], vch[:, 0:DH],
                        bvb_sb[:, 0:DH]))
                    kv_tail.append(nc.vector.tensor_add(
                        vp1[:, kc * VW:kc * VW + DH], vch[:, DH:128],
                        bvb_sb[:, DH:128]))
            _pe_touch(nc, kv_tail)

            # ---------------- attention (+ Q proj, out proj interleaved) ----
            with tc.tile_pool(name="attn", bufs=1) as ap, \
                 tc.tile_pool(name="attnps", bufs=1, space="PSUM") as pb:

                # Act-independent PE matmul filler: emitted a few per key
                # chunk inside the attention loop so the PE never idles
                # (idling drops it out of the fast p-state) while the Act
                # engine works through the exp backlog.
                def q_proj_filler(qt, j):
                    qp = pb.tile([128, TT], F32, tag="qp", bufs=1)

                    def emit(ck):
                        nc.tensor.matmul(
                            qp[:, :],
                            wq_sb[:, ck * QD + j * 128:
                                  ck * QD + (j + 1) * 128],
                            xT_sb[:, ck * TOKS + qt * TT:
                                  ck * TOKS + (qt + 1) * TT],
                            start=(ck == 0), stop=(ck == NCK - 1))
                        if ck == NCK - 1:
                            c0 = j * TOKS + qt * TT
                            nc.vector.tensor_scalar_add(
                                qT[:, c0:c0 + TT],
                                qp[:, :], bq_sb[:, j:j + 1])
                    return [lambda ck=ck: emit(ck) for ck in range(NCK)]

                oT_tiles = {}

                def out_proj_filler(qt, m, oT_sb, op_tag="op"):
                    # one 128-token row block (m) of out: 4 column groups (n)
                    # of 4 j-contraction matmuls each, staged into a full
                    # 2048-col bf16 row tile, then one whole-row DMA (big
                    # descriptors: DMA queues are descriptor-rate-bound).
                    fns = []
                    state = {}

                    def emit(n, jj):
                        if jj == 0:
                            if op_tag == "S":
                                st = pb.tile([128, 2 * QTILE], F32, tag="S",
                                             bufs=2, name="opS")
                                state["op"] = st[:, 0:512]
                            else:
                                state["op"] = pb.tile([128, 512], F32,
                                                      tag="op", bufs=1,
                                                      name="op")
                            if n == 0:
                                state["osb"] = ap.tile([128, D], BF16,
                                                       tag="osb", bufs=2,
                                                       name="osb")
                        nc.tensor.matmul(
                            state["op"][:, :],
                            oT_sb[:, jj * QTILE + m * 128:
                                  jj * QTILE + (m + 1) * 128],
                            wo_sb[:, jj * D + n * 512:
                                  jj * D + (n + 1) * 512],
                            start=(jj == 0), stop=(jj == 3))
                        if jj == 3:
                            nc.vector.tensor_copy(
                                state["osb"][:, n * 512:(n + 1) * 512],
                                state["op"][:, :])
                            if n == 3:
                                nc.sync.dma_start(
                                    out=out[qt * QTILE + m * 128:
                                            qt * QTILE + (m + 1) * 128, :],
                                    in_=state["osb"][:, :])
                    for n in range(4):
                        fns += [lambda n=n, jj=jj: emit(n, jj)
                                for jj in range(4)]
                    return fns

                def drain(filler, k):
                    for _ in range(k):
                        if filler:
                            filler.pop(0)()

                vps = (vp0, vp1)
                # deferred PE-part of the softmax normalize: emitted at the
                # START of the next (qt, j) iteration, after its first two
                # S chunks, so the PE has work while the Act engine computes
                # the reciprocal (Ln + Exp) of this j's denominators
                pending_norm = [None]

                for qt in range(NQT):
                    oT_sb = ap.tile([128, 4 * QTILE], BF16, tag="oTsb", bufs=2)
                    oT_tiles[qt] = oT_sb
                    for j in range(4):
                        fa = out_proj_filler(qt - 1, j, oT_tiles[qt - 1]) \
                            if qt > 0 else []
                        fb = q_proj_filler(qt + 1, j) if qt + 1 < NQT else []
                        # interleave the two filler streams so the op and qp
                        # PSUM banks see alternating pressure
                        filler = [f for pair in
                                  zip(fa, fb) for f in pair] if fa and fb \
                            else (fa or fb)

                        opj0 = pb.tile([VW, QTILE], F32, tag="o0", bufs=1)
                        opj1 = pb.tile([VW, QTILE], F32, tag="o1", bufs=1)
                        opj = [opj0, opj1]
                        Es = {}

                        def s_exp(kc):
                            S = pb.tile([128, 2 * QTILE], F32, tag="S", bufs=2)
                            for half in range(2):
                                nc.tensor.matmul(
                                    S[:, half * QTILE:(half + 1) * QTILE],
                                    kT[half * 64:(half + 1) * 64,
                                       kc * 128:(kc + 1) * 128],
                                    qT[half * 64:(half + 1) * 64,
                                       j * TOKS + qt * QTILE:
                                       j * TOKS + (qt + 1) * QTILE],
                                    start=True, stop=True)
                            E = ap.tile([128, 2 * QTILE], BF16, tag="E",
                                        bufs=4)
                            nc.scalar.activation(
                                E[:, :], S[:, :], AF.Exp, scale=SCALE)
                            Es[kc] = E

                        def pv(kc):
                            E = Es.pop(kc)
                            for half in range(2):
                                nc.tensor.matmul(
                                    opj[half][:, :],
                                    vps[half][:, kc * VW:(kc + 1) * VW],
                                    E[:, half * QTILE:(half + 1) * QTILE],
                                    start=(kc == 0), stop=(kc == NKC - 1))

                        # lag-2 pipeline: pv(kc) consumes exp output a full
                        # two chunks after its S matmuls, so the PE never
                        # waits on the Act engine in steady state
                        s_exp(0)
                        s_exp(1)
                        # at most 3 units before the deferred normalize: the
                        # 4th unit of an out-proj group reads the oT column
                        # that pending_norm is about to write
                        drain(filler, 2)
                        if pending_norm[0] is not None:
                            pending_norm[0]()
                            pending_norm[0] = None
                        drain(filler, 2)
                        for kc in range(2, NKC):
                            s_exp(kc)
                            pv(kc - 2)
                            # keep a filler reserve to cover the reciprocal
                            # latency window after pv(15)
                            drain(filler, 2 if len(filler) > 6 else 1)
                        pv(NKC - 2)
                        drain(filler, 2)
                        pv(NKC - 1)

                        # normalize: reciprocal of the denominator rows (PSUM
                        # partition 64 of each opj bank) as exp(-ln(d)) on the
                        # Act engine (ln+exp share one act table; DVE's true
                        # reciprocal is a 3.4us multi-pass op), written
                        # directly as bf16, broadcast across partitions via
                        # PE ones-matmul into a recycled S-pool slot, then
                        # multiply.
                        r32 = ap.tile([VW, 2 * QTILE], F32, tag="r", bufs=2)
                        rb = ap.tile([VW, 2 * QTILE], BF16, tag="rb", bufs=2)
                        nc.scalar.activation(r32[64:65, 0:QTILE],
                                             opj[0][64:65, :],
                                             AF.Ln, bias=zer_sb[64:65, 0:1])
                        nc.scalar.activation(r32[64:65, QTILE:],
                                             opj[1][64:65, :],
                                             AF.Ln, bias=zer_sb[64:65, 0:1])
                        nc.scalar.activation(rb[64:65, :], r32[64:65, :],
                                             AF.Exp, scale=-1.0,
                                             bias=zer_sb[64:65, 0:1])
                        drain(filler, len(filler))

                        def norm_pe(j=j, opj=opj, rb=rb, oT_sb=oT_sb):
                            bcs = ap.tile([64, 2 * QTILE], F32, tag="bcs",
                                          bufs=2)
                            nm1 = ap.tile([64, QTILE], BF16, tag="nm1",
                                          bufs=2)
                            bct = pb.tile([128, 2 * QTILE], F32, tag="S",
                                          bufs=2)
                            nc.tensor.matmul(
                                bct[0:64, 0:QTILE], ones_b[64:65, 0:DH],
                                rb[64:65, 0:QTILE], start=True, stop=True)
                            nc.tensor.matmul(
                                bct[0:64, QTILE:], ones_b[64:65, 0:DH],
                                rb[64:65, QTILE:], start=True, stop=True)
                            nc.vector.tensor_copy(bcs[:, :], bct[0:64, :])
                            nc.vector.tensor_mul(
                                oT_sb[0:64, j * QTILE:(j + 1) * QTILE],
                                opj[0][0:64, :], bcs[:, 0:QTILE])
                            # half1 numerator sits at partitions 0:64 of its
                            # PSUM bank; multiply in place then partition-
                            # shift the bf16 result into oT rows 64:128 with
                            # an SB->SB DMA.
                            nc.vector.tensor_mul(
                                nm1[:, :], opj[1][0:64, :], bcs[:, QTILE:])
                            nc.sync.dma_start(
                                out=oT_sb[64:128, j * QTILE:(j + 1) * QTILE],
                                in_=nm1[:, :])
                        pending_norm[0] = norm_pe
                    if qt == NQT - 1:
                        pending_norm[0]()
                        pending_norm[0] = None
                        # tail: alternate op banks with free S-pool banks so
                        # the PSUM copy of one row block overlaps the
                        # matmuls of the next
                        for m in range(4):
                            for fn in out_proj_filler(
                                    qt, m, oT_sb,
                                    op_tag="S" if m % 2 else "op"):
                                fn()
    return nc


def _prep_inputs(x, Wq, bq, Wk, bk, Wv, bv, Wo, bo):
    """Build the 8 per-core input maps (host-side shard + transpose)."""
    f = np.float32
    bf = ml_dtypes.bfloat16
    x = np.asarray(x, f)
    Wq, bq = np.asarray(Wq, f), np.asarray(bq, f)
    Wk, bk = np.asarray(Wk, f), np.asarray(bk, f)
    Wv, bv = np.asarray(Wv, f), np.asarray(bv, f)
    Wo = np.asarray(Wo, f)
    # per-core head-dim permutation within the group's 512 q dims
    perm = np.concatenate([
        np.arange(h * DH, (h + 1) * DH) for h in HEAD_ORDER])
    xT_b = [np.ascontiguousarray(x[b].T.astype(bf)) for b in range(2)]
    in_maps = []
    for c in range(8):
        b, g = divmod(c, 4)
        wq_g = Wq[g * QD:(g + 1) * QD, :][perm, :]     # (512, 2048)
        bq_g = bq[g * QD:(g + 1) * QD][perm]
        wo_g = Wo[:, g * QD:(g + 1) * QD].T[perm, :]   # (512, 2048)
        def pack(wT, nchunks):
            # [nchunks*128, cols] -> SBUF layout [128, nchunks*cols]
            w = np.asarray(wT, np.float32)
            cols = w.shape[1]
            w = w.reshape(nchunks, 128, cols).transpose(1, 0, 2)
            return np.ascontiguousarray(w.reshape(128, nchunks * cols)
                                        .astype(bf))

        in_maps.append({
            "xT": xT_b[b],
            "wqP": pack(wq_g.T, NCK),
            "wkP": pack(Wk[g * 128:(g + 1) * 128, :].T, NCK),
            "wvP": pack(Wv[g * 128:(g + 1) * 128, :].T, NCK),
            "woP": pack(wo_g, 4),
            "bq4": np.ascontiguousarray(bq_g.reshape(4, 128).T),
            "bkT": np.ascontiguousarray(bk[g * 128:(g + 1) * 128, None]),
            "bvb": np.ascontiguousarray(
                np.broadcast_to(bv[g * 128:(g + 1) * 128], (128, 128)).copy()),
        })
    return in_maps


def run(inputs, trace=False, **kw):
    nc = _build()
    _orig_tjb = nc.to_json_bytes
    nc.to_json_bytes = lambda: _split_matmul_waits(_orig_tjb())
    in_maps = _prep_inputs(**inputs)
    res = run_bass_kernel_spmd(nc, in_maps, list(range(8)), trace=trace, **kw)
    bo = np.asarray(inputs["bo"], np.float32)
    y = np.empty((2, TOKS, D), np.float32)
    for b in range(2):
        acc = res.results[4 * b]["out"].astype(np.float32)
        for g in range(1, 4):
            acc = acc + res.results[4 * b + g]["out"].astype(np.float32)
        y[b] = acc + bo[None, :]
    return y, res


def kernel(**inputs):
    y, _ = run(inputs, trace=False)
    return y


# revision 55
# speedup vs baseline: 1.0083x; 1.0083x over previous
"""GQA kernel for trn2, 8 NeuronCores — bf16 PE pipeline.

Problem: B=2, N=2048, d_model=2048, 32 q heads / 8 kv heads, d_head=64.
Sharding: batch (2) x head-groups (4): core c = b*4 + g handles batch b and
q heads [8g, 8g+8) (kv heads [2g, 2g+1]).  Each core computes
partial_out = attn_out_g @ Wo[:, cols_g].T ; host sums the 4 group partials
per batch and adds bo.

Key speed choices vs the fp32 baseline:
  * all matmuls in bf16 (1 cycle/row vs 4 for fp32 on the PE)
  * x is transposed to xT on the host -> no on-device transpose phase
  * V' carries a ones column (65th) so the softmax denominator falls out
    of the PV matmul as output partition 64 -- no separate denominator
    matmuls
  * Act engine runs only exp (+ K/Q bias copies); normalize reciprocal is
    broadcast across partitions by gpsimd and multiplied on DVE
  * S -> exp -> PV software-pipelined one key-chunk deep; out-proj and
    next-tile Q-proj matmuls are interleaved as Act-independent PE filler

Per-core layouts (bf16 unless noted):
  xT_sb [128, 16*2048]  d_model chunk ck at free [ck*2048, +2048); free=toks
  qT  [128, 4*2048]  chunk j holds heads (j, j+4): partitions 0:64 = head j
                     dims, 64:128 = head j+4 dims; free = tokens.
  kT  [128, 2048]    partitions 0:64 = kv0 k-dims, 64:128 = kv1 k-dims.
  vp0/vp1 [128, 16*65]  V' per kv head: partitions = keys (chunk kc at
                     free [kc*65, +65)), free = 64 v dims + ones col.
  S.T computed as [keys, q] (exp is safe unnormalized: |scores/8| < ~6).
"""

import numpy as np
import ml_dtypes

import concourse.bass as bass
import concourse.mybir as mybir
from concourse.tile import TileContext, add_dep_helper
from concourse.bass_utils import run_bass_kernel_spmd


def _split_matmul_waits(bir_bytes):
    """Walrus in this toolchain allows only ONE sync wait per Matmult.

    For any matmul carrying N>1 waits, insert a PE NoOp immediately
    before it holding the first N-1 waits; the matmul keeps the last.
    The NoOp precedes the matmul in the PE stream, so ordering
    semantics are identical.
    """
    import json as _json
    bir = _json.loads(bir_bytes)
    n = 0
    for f in bir["functions"]:
        for b in f["blocks"]:
            out = []
            for i in b["instructions"]:
                si = i.get("sync_info") if isinstance(i, dict) else None
                eng = i.get("engine") if isinstance(i, dict) else None
                if (si and len(si.get("on_wait", [])) > 1
                        and eng and eng != "Unassigned"):
                    waits = si["on_wait"]
                    for w in waits[:-1]:
                        out.append({
                            "debug": i.get("debug", 0),
                            "engine": eng,
                            "ins": [], "outs": [],
                            "name": "%s-w%d" % (i["name"], n),
                            "opcode": "NoOp",
                            "sync_info": {"on_update": [], "on_wait": [w]},
                        })
                        n += 1
                    si["on_wait"] = waits[-1:]
                out.append(i)
            b["instructions"] = out
    return _json.dumps(bir).encode()


def _pe_touch(nc, producers):
    """Advance PE's vector clock past each producer, one sem at a time.

    This walrus build allows at most ONE sync wait per Matmult; a matmul
    whose inputs come from two not-yet-observed semaphores fails codegen
    ("Too many sync wait commands").  A PE nop with a single sync dep
    absorbs one semaphore each, so real matmuls only ever add one wait.
    """
    for p in producers:
        n = nc.tensor.nop()
        add_dep_helper(n.ins, p.ins, sync=True, reason="pe-wait-absorber")


F32 = mybir.dt.float32
BF16 = mybir.dt.bfloat16
FP8 = mybir.dt.float8e4
AF = mybir.ActivationFunctionType
DBLROW = mybir.MatmulPerfMode.DoubleRow

D = 2048      # d_model
TOKS = 2048   # tokens per batch
QD = 512      # q dims per core
DH = 64
NCK = 16      # d_model chunks of 128
TT = 512      # token tile for projections
NTT = TOKS // TT
QTILE = 512
NQT = TOKS // QTILE
NKC = TOKS // 128   # key chunks of 128
VW = DH + 1   # V' chunk width incl. ones column
SCALE = DH ** -0.5  # 0.125

# local head order within a core: chunk j holds heads (j, j+4)
HEAD_ORDER = [0, 4, 1, 5, 2, 6, 3, 7]


def _build():
    nc = bass.Bass()
    # weights come pre-packed in SBUF layout (partition-major) so each loads
    # with a single DMA of long descriptors; DMA queues are descriptor-rate
    # bound, not bandwidth bound
    xT = nc.declare_dram_parameter("xT", [D, TOKS], BF16, isOutput=False)
    wqP = nc.declare_dram_parameter("wqP", [128, NCK * QD], BF16,
                                    isOutput=False)
    wkP = nc.declare_dram_parameter("wkP", [128, NCK * 128], BF16,
                                    isOutput=False)
    wvP = nc.declare_dram_parameter("wvP", [128, NCK * 128], BF16,
                                    isOutput=False)
    woP = nc.declare_dram_parameter("woP", [128, 4 * D], BF16, isOutput=False)
    bq4 = nc.declare_dram_parameter("bq4", [128, 4], F32, isOutput=False)
    bkT = nc.declare_dram_parameter("bkT", [128, 1], F32, isOutput=False)
    bvb = nc.declare_dram_parameter("bvb", [128, 128], F32, isOutput=False)
    out = nc.declare_dram_parameter("out", [TOKS, D], BF16, isOutput=True)

    with TileContext(nc) as tc:
        with tc.tile_pool(name="persist", bufs=1) as pp:
            wq_sb = pp.tile([128, NCK * QD], BF16, tag="wq")
            wk_sb = pp.tile([128, NCK * 128], BF16, tag="wk")
            wv_sb = pp.tile([128, NCK * 128], BF16, tag="wv")
            wo_sb = pp.tile([128, 4 * D], BF16, tag="wo")
            xT_sb = pp.tile([128, NCK * TOKS], BF16, tag="xTs")
            qT = pp.tile([128, 4 * TOKS], BF16, tag="qT")
            kT = pp.tile([128, TOKS], BF16, tag="kT")
            vp0 = pp.tile([128, NKC * VW], BF16, tag="vp0")
            vp1 = pp.tile([128, NKC * VW], BF16, tag="vp1")
            bq_sb = pp.tile([128, 4], F32, tag="bq")
            bk_sb = pp.tile([128, 1], F32, tag="bk")
            bvb_sb = pp.tile([128, 128], F32, tag="bvb")
            ones_b = pp.tile([65, DH], BF16, tag="ones")
            zer_sb = pp.tile([65, 1], F32, tag="zer")

            const_loads = []
            # weights needed earliest first: wk/wq (K and Q0 streams) loaded
            # per d_model chunk, interleaved with the xT chunks they pair
            # with, so the streaming projections chase the DMAs
            const_loads.append(nc.sync.dma_start(out=bk_sb[:, :], in_=bkT[:, :]))
            const_loads.append(nc.sync.dma_start(out=bq_sb[:, :], in_=bq4[:, :]))
            for ck in range(NCK):
                const_loads.append(nc.sync.dma_start(
                    out=wk_sb[:, ck * 128:(ck + 1) * 128],
                    in_=wkP[:, ck * 128:(ck + 1) * 128]))
                const_loads.append(nc.sync.dma_start(
                    out=wq_sb[:, ck * QD:(ck + 1) * QD],
                    in_=wqP[:, ck * QD:(ck + 1) * QD]))
                const_loads.append(nc.sync.dma_start(
                    out=xT_sb[:, ck * TOKS:(ck + 1) * TOKS],
                    in_=xT[ck * 128:(ck + 1) * 128, :]))
            const_loads.append(nc.sync.dma_start(out=bvb_sb[:, :], in_=bvb[:, :]))
            for q in range(4):
                const_loads.append(nc.sync.dma_start(
                    out=wv_sb[:, q * 512:(q + 1) * 512],
                    in_=wvP[:, q * 512:(q + 1) * 512]))
            for j in range(4):
                const_loads.append(nc.sync.dma_start(
                    out=wo_sb[:, j * D:(j + 1) * D],
                    in_=woP[:, j * D:(j + 1) * D]))
            # ones columns of V': preset whole vp tiles to 1.0; the V adds
            # later overwrite the 64 data columns of each chunk
            const_loads.append(nc.vector.memset(vp0[:, :], 1.0))
            const_loads.append(nc.vector.memset(vp1[:, :], 1.0))
            const_loads.append(nc.vector.memset(ones_b[:, :], 1.0))
            const_loads.append(nc.vector.memset(zer_sb[:, :], 0.0))
            _pe_touch(nc, const_loads)

            # ---------------- K and first-tile-Q projections ----------------
            # Streamed per d_model chunk (outer ck) so the matmuls chase the
            # xT DMAs chunk-by-chunk instead of waiting for the whole load;
            # all 4 token-tile accumulators of each live concurrently.
            kv_tail = []
            with tc.tile_pool(name="kqps", bufs=1, space="PSUM") as kqp:
                kps, qps = [], []
                for tt in range(NTT):
                    kp_t = kqp.tile([128, TT], F32, tag="kp", bufs=NTT,
                                    name="kp")
                    qp_t = kqp.tile([128, TT], F32, tag="qp0", bufs=NTT,
                                    name="qp0")
                    kps.append(kp_t)
                    qps.append(qp_t)
                for ck in range(NCK):
                    for tt in range(NTT):
                        nc.tensor.matmul(
                            kps[tt][:, :],
                            wk_sb[:, ck * 128:(ck + 1) * 128],
                            xT_sb[:, ck * TOKS + tt * TT:
                                  ck * TOKS + (tt + 1) * TT],
                            start=(ck == 0), stop=(ck == NCK - 1))
                        nc.tensor.matmul(
                            qps[tt][:, :],
                            wq_sb[:, ck * QD + tt * 128:
                                  ck * QD + (tt + 1) * 128],
                            xT_sb[:, ck * TOKS:ck * TOKS + TT],
                            start=(ck == 0), stop=(ck == NCK - 1))
                for tt in range(NTT):
                    kv_tail.append(nc.scalar.activation(
                        kT[:, tt * TT:(tt + 1) * TT], kps[tt][:, :],
                        AF.Identity, bias=bk_sb[:, 0:1]))
                    kv_tail.append(nc.vector.tensor_scalar_add(
                        qT[:, tt * TOKS:tt * TOKS + TT],
                        qps[tt][:, :], bq_sb[:, tt:tt + 1]))
            # ---------------- V projection ----------------
            with tc.tile_pool(name="vps", bufs=1, space="PSUM") as kvp:
                for kc in range(NKC):
                    vch = kvp.tile([128, 128], F32, tag="vch", bufs=4)
                    for ck in range(NCK):
                        nc.tensor.matmul(
                            vch[:, :],
                            xT_sb[:, ck * TOKS + kc * 128:
                                  ck * TOKS + (kc + 1) * 128],
                            wv_sb[:, ck * 128:(ck + 1) * 128],
                            start=(ck == 0), stop=(ck == NCK - 1))
                    kv_tail.append(nc.vector.tensor_add(
                        vp0[:, kc * VW:kc * VW + DH], vch[:, 0:DH],
                        bvb_sb[:, 0:DH]))
                    kv_tail.append(nc.vector.tensor_add(
                        vp1[:, kc * VW:kc * VW + DH], vch[:, DH:128],
                        bvb_sb[:, DH:128]))
            _pe_touch(nc, kv_tail)

            # ---------------- attention (+ Q proj, out proj interleaved) ----
            with tc.tile_pool(name="attn", bufs=1) as ap, \
                 tc.tile_pool(name="attnps", bufs=1, space="PSUM") as pb:

                # Act-independent PE matmul filler: emitted a few per key
                # chunk inside the attention loop so the PE never idles
                # (idling drops it out of the fast p-state) while the Act
                # engine works through the exp backlog.
                def q_proj_filler(qt, j):
                    qp = pb.tile([128, TT], F32, tag="qp", bufs=1)

                    def emit(ck):
                        nc.tensor.matmul(
                            qp[:, :],
                            wq_sb[:, ck * QD + j * 128:
                                  ck * QD + (j + 1) * 128],
                            xT_sb[:, ck * TOKS + qt * TT:
                                  ck * TOKS + (qt + 1) * TT],
                            start=(ck == 0), stop=(ck == NCK - 1))
                        if ck == NCK - 1:
                            c0 = j * TOKS + qt * TT
                            nc.vector.tensor_scalar_add(
                                qT[:, c0:c0 + TT],
                                qp[:, :], bq_sb[:, j:j + 1])
                    return [lambda ck=ck: emit(ck) for ck in range(NCK)]

                oT_tiles = {}

                def out_proj_filler(qt, m, oT_sb, op_tag="op"):
                    # one 128-token row block (m) of out: 4 column groups (n)
                    # of 4 j-contraction matmuls each, staged into a full
                    # 2048-col bf16 row tile, then one whole-row DMA (big
                    # descriptors: DMA queues are descriptor-rate-bound).
                    fns = []
                    state = {}

                    def emit(n, jj):
                        if jj == 0:
                            if op_tag == "S":
                                st = pb.tile([128, 2 * QTILE], F32, tag="S",
                                             bufs=2, name="opS")
                                state["op"] = st[:, 0:512]
                            else:
                                state["op"] = pb.tile([128, 512], F32,
                                                      tag="op", bufs=1,
                                                      name="op")
                            if n == 0:
                                state["osb"] = ap.tile([128, D], BF16,
                                                       tag="osb", bufs=2,
                                                       name="osb")
                        nc.tensor.matmul(
                            state["op"][:, :],
                            oT_sb[:, jj * QTILE + m * 128:
                                  jj * QTILE + (m + 1) * 128],
                            wo_sb[:, jj * D + n * 512:
                                  jj * D + (n + 1) * 512],
                            start=(jj == 0), stop=(jj == 3))
                        if jj == 3:
                            nc.vector.tensor_copy(
                                state["osb"][:, n * 512:(n + 1) * 512],
                                state["op"][:, :])
                            if n == 3:
                                nc.sync.dma_start(
                                    out=out[qt * QTILE + m * 128:
                                            qt * QTILE + (m + 1) * 128, :],
                                    in_=state["osb"][:, :])
                    for n in range(4):
                        fns += [lambda n=n, jj=jj: emit(n, jj)
                                for jj in range(4)]
                    return fns

                def drain(filler, k):
                    for _ in range(k):
                        if filler:
                            filler.pop(0)()

                vps = (vp0, vp1)
                # deferred PE-part of the softmax normalize: emitted at the
                # START of the next (qt, j) iteration, after its first two
                # S chunks, so the PE has work while the Act engine computes
                # the reciprocal (Ln + Exp) of this j's denominators
                pending_norm = [None]

                for qt in range(NQT):
                    oT_sb = ap.tile([128, 4 * QTILE], BF16, tag="oTsb", bufs=2)
                    oT_tiles[qt] = oT_sb
                    for j in range(4):
                        fa = out_proj_filler(qt - 1, j, oT_tiles[qt - 1]) \
                            if qt > 0 else []
                        fb = q_proj_filler(qt + 1, j) if qt + 1 < NQT else []
                        # interleave the two filler streams so the op and qp
                        # PSUM banks see alternating pressure
                        filler = [f for pair in
                                  zip(fa, fb) for f in pair] if fa and fb \
                            else (fa or fb)

                        opj0 = pb.tile([VW, QTILE], F32, tag="o0", bufs=1)
                        opj1 = pb.tile([VW, QTILE], F32, tag="o1", bufs=1)
                        opj = [opj0, opj1]
                        Es = {}

                        def s_exp(kc):
                            S = pb.tile([128, 2 * QTILE], F32, tag="S", bufs=2)
                            for half in range(2):
                                nc.tensor.matmul(
                                    S[:, half * QTILE:(half + 1) * QTILE],
                                    kT[half * 64:(half + 1) * 64,
                                       kc * 128:(kc + 1) * 128],
                                    qT[half * 64:(half + 1) * 64,
                                       j * TOKS + qt * QTILE:
                                       j * TOKS + (qt + 1) * QTILE],
                                    start=True, stop=True)
                            E = ap.tile([128, 2 * QTILE], BF16, tag="E",
                                        bufs=4)
                            nc.scalar.activation(
                                E[:, :], S[:, :], AF.Exp, scale=SCALE)
                            Es[kc] = E

                        def pv(kc):
                            E = Es.pop(kc)
                            for half in range(2):
                                nc.tensor.matmul(
                                    opj[half][:, :],
                                    vps[half][:, kc * VW:(kc + 1) * VW],
                                    E[:, half * QTILE:(half + 1) * QTILE],
                                    start=(kc == 0), stop=(kc == NKC - 1))

                        # lag-2 pipeline: pv(kc) consumes exp output a full
                        # two chunks after its S matmuls, so the PE never
                        # waits on the Act engine in steady state
                        s_exp(0)
                        s_exp(1)
                        # at most 3 units before the deferred normalize: the
                        # 4th unit of an out-proj group reads the oT column
                        # that pending_norm is about to write
                        drain(filler, 2)
                        if pending_norm[0] is not None:
                            pending_norm[0]()
                            pending_norm[0] = None
                        drain(filler, 2)
                        for kc in range(2, NKC):
                            s_exp(kc)
                            pv(kc - 2)
                            drain(filler, 2)
                        pv(NKC - 2)
                        drain(filler, 3)
                        pv(NKC - 1)

                        # normalize: reciprocal of the denominator rows (PSUM
                        # partition 64 of each opj bank) as exp(-ln(d)) on the
                        # Act engine (ln+exp share one act table; DVE's true
                        # reciprocal is a 3.4us multi-pass op), written
                        # directly as bf16, broadcast across partitions via
                        # PE ones-matmul into a recycled S-pool slot, then
                        # multiply.
                        r32 = ap.tile([VW, 2 * QTILE], F32, tag="r", bufs=2)
                        rb = ap.tile([VW, 2 * QTILE], BF16, tag="rb", bufs=2)
                        nc.scalar.activation(r32[64:65, 0:QTILE],
                                             opj[0][64:65, :],
                                             AF.Ln, bias=zer_sb[64:65, 0:1])
                        nc.scalar.activation(r32[64:65, QTILE:],
                                             opj[1][64:65, :],
                                             AF.Ln, bias=zer_sb[64:65, 0:1])
                        nc.scalar.activation(rb[64:65, :], r32[64:65, :],
                                             AF.Exp, scale=-1.0,
                                             bias=zer_sb[64:65, 0:1])
                        drain(filler, len(filler))

                        def norm_pe(j=j, opj=opj, rb=rb, oT_sb=oT_sb):
                            bcs = ap.tile([64, 2 * QTILE], F32, tag="bcs",
                                          bufs=2)
                            nm1 = ap.tile([64, QTILE], BF16, tag="nm1",
                                          bufs=2)
                            bct = pb.tile([128, 2 * QTILE], F32, tag="S",
                                          bufs=2)
                            nc.tensor.matmul(
                                bct[0:64, 0:QTILE], ones_b[64:65, 0:DH],
                                rb[64:65, 0:QTILE], start=True, stop=True)
                            nc.tensor.matmul(
                                bct[0:64, QTILE:], ones_b[64:65, 0:DH],
                                rb[64:65, QTILE:], start=True, stop=True)
                            nc.vector.tensor_copy(bcs[:, :], bct[0:64, :])
                            nc.vector.tensor_mul(
                                oT_sb[0:64, j * QTILE:(j + 1) * QTILE],
                                opj[0][0:64, :], bcs[:, 0:QTILE])
                            # half1 numerator sits at partitions 0:64 of its
                            # PSUM bank; multiply in place then partition-
                            # shift the bf16 result into oT rows 64:128 with
                            # an SB->SB DMA.
                            nc.vector.tensor_mul(
                                nm1[:, :], opj[1][0:64, :], bcs[:, QTILE:])
                            nc.sync.dma_start(
                                out=oT_sb[64:128, j * QTILE:(j + 1) * QTILE],
                                in_=nm1[:, :])
                        pending_norm[0] = norm_pe
                    if qt == NQT - 1:
                        pending_norm[0]()
                        pending_norm[0] = None
                        # tail: alternate op banks with free S-pool banks so
                        # the PSUM copy of one row block overlaps the
                        # matmuls of the next
                        for m in range(4):
                            for fn in out_proj_filler(
                                    qt, m, oT_sb,
                                    op_tag="S" if m % 2 else "op"):
                                fn()
    return nc


def _prep_inputs(x, Wq, bq, Wk, bk, Wv, bv, Wo, bo):
    """Build the 8 per-core input maps (host-side shard + transpose)."""
    f = np.float32
    bf = ml_dtypes.bfloat16
    x = np.asarray(x, f)
    Wq, bq = np.asarray(Wq, f), np.asarray(bq, f)
    Wk, bk = np.asarray(Wk, f), np.asarray(bk, f)
    Wv, bv = np.asarray(Wv, f), np.asarray(bv, f)
    Wo = np.asarray(Wo, f)
    # per-core head-dim permutation within the group's 512 q dims
    perm = np.concatenate([
        np.arange(h * DH, (h + 1) * DH) for h in HEAD_ORDER])
    xT_b = [np.ascontiguousarray(x[b].T.astype(bf)) for b in range(2)]
    in_maps = []
    for c in range(8):
        b, g = divmod(c, 4)
        wq_g = Wq[g * QD:(g + 1) * QD, :][perm, :]     # (512, 2048)
        bq_g = bq[g * QD:(g + 1) * QD][perm]
        wo_g = Wo[:, g * QD:(g + 1) * QD].T[perm, :]   # (512, 2048)
        def pack(wT, nchunks):
            # [nchunks*128, cols] -> SBUF layout [128, nchunks*cols]
            w = np.asarray(wT, np.float32)
            cols = w.shape[1]
            w = w.reshape(nchunks, 128, cols).transpose(1, 0, 2)
            return np.ascontiguousarray(w.reshape(128, nchunks * cols)
                                        .astype(bf))

        in_maps.append({
            "xT": xT_b[b],
            "wqP": pack(wq_g.T, NCK),
            "wkP": pack(Wk[g * 128:(g + 1) * 128, :].T, NCK),
            "wvP": pack(Wv[g * 128:(g + 1) * 128, :].T, NCK),
            "woP": pack(wo_g, 4),
            "bq4": np.ascontiguousarray(bq_g.reshape(4, 128).T),
            "bkT": np.ascontiguousarray(bk[g * 128:(g + 1) * 128, None]),
            "bvb": np.ascontiguousarray(
                np.broadcast_to(bv[g * 128:(g + 1) * 128], (128, 128)).copy()),
        })
    return in_maps


def run(inputs, trace=False, **kw):
    nc = _build()
    _orig_tjb = nc.to_json_bytes
    nc.to_json_bytes = lambda: _split_matmul_waits(_orig_tjb())
    in_maps = _prep_inputs(**inputs)
    res = run_bass_kernel_spmd(nc, in_maps, list(range(8)), trace=trace, **kw)
    bo = np.asarray(inputs["bo"], np.float32)
    y = np.empty((2, TOKS, D), np.float32)
    for b in range(2):
        acc = res.results[4 * b]["out"].astype(np.float32)
        for g in range(1, 4):
            acc = acc + res.results[4 * b + g]["out"].astype(np.float32)
        y[b] = acc + bo[None, :]
    return y, res


def kernel(**inputs):
    y, _ = run(inputs, trace=False)
    return y
